# revision 28
# baseline (speedup 1.0000x reference)
"""Trainium2 Bass kernel for nn_AdvancedActorCritic (dense_transformer).

Strategy:
  - 8 NeuronCores, SPMD: cores 0-3 run the actor net, cores 4-7 the critic
    net (its output layer zero-padded to the actor's shape so one program
    serves both).  Each core gets a 4096-row batch shard of obs.
  - seq_len == 1  =>  softmax over a single key is identity  =>  attention
    reduces to wo(wv(x)); wq/wk are dead code.
  - Activation layout: [features-on-partitions, batch-on-free].  Weights are
    pre-transposed on the host, so every linear layer is a weight-stationary
    matmul chain with N=512 moving tiles.
  - LayerNorm: partition-dim sums via ones-matmul on the PE (replicated over
    128 partitions), var = E[x^2]-m^2, ACT Sqrt + DVE reciprocal + one
    Newton-Raphson step for rsqrt, apply fused via scalar_tensor_tensor.
  - GELU: ACT Gelu_apprx_tanh (the exact tanh-approx formula of the ref).
"""

import sys

sys.path.insert(0, "/opt/trn_rl_repo")

from contextlib import ExitStack

import numpy as np

import concourse.bass as bass
import concourse.mybir as mybir
import concourse.tile as tile
from concourse import bacc

FP = mybir.dt.float32
F16 = mybir.dt.float16
MMDT = F16       # dtype for matmul operands (weights + activations)
AF = mybir.ActivationFunctionType
OP = mybir.AluOpType

OBS = 384
D0 = 512
DFF = 2048
DH1 = 256
AOUT = 24
EPS = 1e-5
BATCH = 16384
NCORES = 8
B_SHARD = BATCH // 4  # 4096 rows per core (actor on 0-3, critic on 4-7)
BT = 512              # matmul moving-tile width (fp32 PSUM bank)
CHUNK = 2048          # batch columns resident in SBUF per pass

# ---------------------------------------------------------------------------
# parameter-vector packing spec: (key, length).  Packed column-major into a
# [128, NCOL] f32 tensor; vector v occupies cols [off, off+len/128) with
# column j = v[128j:128j+128].
# ---------------------------------------------------------------------------
P_SPEC = [
    ("b_in", 512), ("g_in", 512), ("be_in", 512),
    ("bv1", 512), ("bo1", 512), ("g11", 512), ("be11", 512),
    ("bf11", 2048), ("bf21", 2048), ("bf31", 512), ("g21", 512), ("be21", 512),
    ("bv2", 512), ("bo2", 512), ("g12", 512), ("be12", 512),
    ("bf12", 2048), ("bf22", 2048), ("bf32", 512), ("g22", 512), ("be22", 512),
    ("bh0", 512), ("gh0", 512), ("beh0", 512), ("bg0", 512), ("bt0", 512),
    ("bh1", 256), ("gh1", 256), ("beh1", 256), ("bg1", 256), ("bt1", 256),
    ("bout", 128),
]
P_COL = {}
_c = 0
for _k, _d in P_SPEC:
    P_COL[_k] = _c
    _c += _d // 128
NCOL = _c

W_SHAPES = {
    "w_in": (OBS, D0),
    "w_v1": (D0, D0), "w_o1": (D0, D0),
    "w_f11": (D0, DFF), "w_f21": (D0, DFF), "w_f31": (DFF, D0),
    "w_v2": (D0, D0), "w_o2": (D0, D0),
    "w_f12": (D0, DFF), "w_f22": (D0, DFF), "w_f32": (DFF, D0),
    "w_h0": (D0, D0), "w_gt0": (D0, 2 * D0),
    "w_h1": (D0, DH1), "w_gt1": (DH1, 2 * DH1),
    "w_out": (DH1, AOUT),
}


# ---------------------------------------------------------------------------
# kernel program
# ---------------------------------------------------------------------------
def build_nc(b_shard=B_SHARD, chunk=CHUNK):
    nc = bacc.Bacc("TRN2", target_bir_lowering=False, debug=False)

    obsT = nc.dram_tensor("obsT", [OBS, b_shard], MMDT, kind="ExternalInput")
    wd = {
        name: nc.dram_tensor(name, list(shape), MMDT, kind="ExternalInput")
        for name, shape in W_SHAPES.items()
    }
    prm_d = nc.dram_tensor("params", [128, NCOL], FP, kind="ExternalInput")
    out_d = nc.dram_tensor("out", [AOUT, b_shard], FP, kind="ExternalOutput")

    PW = 2 * BT                 # LN batch width: a "pair" of matmul tiles
    nchunks = b_shard // chunk
    npairs = chunk // PW

    with tile.TileContext(nc) as tc, ExitStack() as ctx:
        act = ctx.enter_context(tc.tile_pool(name="act", bufs=9))
        wide = ctx.enter_context(tc.tile_pool(name="wide", bufs=8))
        wf3p = ctx.enter_context(tc.tile_pool(name="wf3p", bufs=16))
        wsm = ctx.enter_context(tc.tile_pool(name="wsm", bufs=8))
        wsm2 = ctx.enter_context(tc.tile_pool(name="wsm2", bufs=8))
        scrw = ctx.enter_context(tc.tile_pool(name="scrw", bufs=7))   # fp32 [128,PW]
        s16 = ctx.enter_context(tc.tile_pool(name="s16", bufs=18))     # fp16 [128,PW]
        scrh = ctx.enter_context(tc.tile_pool(name="scrh", bufs=6))   # fp16 [128,BT]
        obsp = ctx.enter_context(tc.tile_pool(name="obsp", bufs=4))
        cons = ctx.enter_context(tc.tile_pool(name="cons", bufs=1))
        outp = ctx.enter_context(tc.tile_pool(name="outp", bufs=1))
        pmm = ctx.enter_context(
            tc.tile_pool(name="pmm", bufs=2, space=bass.MemorySpace.PSUM))
        pgv = ctx.enter_context(
            tc.tile_pool(name="pgv", bufs=2, space=bass.MemorySpace.PSUM))
        pst = ctx.enter_context(
            tc.tile_pool(name="pst", bufs=2, space=bass.MemorySpace.PSUM))
        pfa = ctx.enter_context(
            tc.tile_pool(name="pfa", bufs=2, space=bass.MemorySpace.PSUM))

        ones = cons.tile([128, 128], MMDT, tag="ones", name="ones")
        nc.vector.memset(ones, 1.0)
        prm = cons.tile([128, NCOL], FP, tag="prm", name="prm")
        nc.sync.dma_start(out=prm, in_=prm_d[:, :])
        eps_t = cons.tile([128, 1], FP, tag="epsb", name="epsb")
        nc.vector.memset(eps_t, EPS)

        def P(key, m=0):
            c = P_COL[key] + m
            return prm[:, c:c + 1]

        def load_w(name, k, c0, cw, pool, tag):
            t = pool.tile([128, cw], MMDT, tag=tag, name="w_" + tag)
            nc.sync.dma_start(out=t, in_=wd[name][128 * k:128 * (k + 1), c0:c0 + cw])
            return t

        def load_layer(name, kdim, cw, pool=wsm, tag="wsm"):
            return [load_w(name, k, 0, cw, pool, tag) for k in range(kdim // 128)]

        def stt(out_ap, in0, scalar, in1, op0, op1):
            nc.vector.scalar_tensor_tensor(out_ap, in0, scalar, in1, op0, op1)

        def act_rsqrt(out_ap, in_ap):
            """out = 1/sqrt(in + EPS) via the ACT spline (direct emission —
            the bass wrapper refuses Rsqrt); one Newton step follows."""
            ins = [nc.scalar.lower_ap(in_ap),
                   nc.scalar.lower_ap(eps_t),
                   mybir.ImmediateValue(dtype=FP, value=1.0),
                   mybir.ImmediateValue(dtype=FP, value=0.0)]
            nc.scalar.add_instruction(
                mybir.InstActivation(
                    name=nc.get_next_instruction_name(),
                    func=AF.Rsqrt,
                    ins=ins,
                    outs=[nc.scalar.lower_ap(out_ap)]))

        def matmul_layer(w_tiles, src, m, n_k, out_ps, cofs=0):
            for k in range(n_k):
                nc.tensor.matmul(
                    out_ps, w_tiles[k][:, cofs + 128 * m: cofs + 128 * (m + 1)],
                    src[k], start=(k == 0), stop=(k == n_k - 1))

        def ln_pair(src, dst, D, gkey, bkey, fuse_gelu=False):
            """Batched layernorm over one pair (PW columns).
            src: list of [128, PW] APs (D//128).  dst: same, or None to
            allocate fp16 pair tiles internally (returned)."""
            n = D // 128
            sqs = []
            for m in range(n):
                sq = s16.tile([128, PW], MMDT, tag="s16", name="sq")
                nc.scalar.activation(sq, src[m], AF.Square)
                sqs.append(sq)
            s1c = scrw.tile([128, PW], FP, tag="scrw", name="s1c")
            s2c = scrw.tile([128, PW], FP, tag="scrw", name="s2c")
            for b2 in range(2):
                sl = slice(b2 * BT, (b2 + 1) * BT)
                ps1 = pst.tile([128, BT], FP, tag="st", name="st")
                for m in range(n):
                    nc.tensor.matmul(ps1, ones[:, :], src[m][:, sl],
                                     start=(m == 0), stop=(m == n - 1))
                nc.scalar.mul(s1c[:, sl], ps1, 1.0 / D)      # mean
                ps2 = pst.tile([128, BT], FP, tag="st", name="st")
                for m in range(n):
                    nc.tensor.matmul(ps2, ones[:, :], sqs[m][:, sl],
                                     start=(m == 0), stop=(m == n - 1))
                nc.scalar.mul(s2c[:, sl], ps2, 1.0 / D)      # E[x^2]
            # centered copies first: frees s1c/src before the rs chain ends.
            # Odd tiles go to GpSimd to keep the DVE chain short.
            xcs = []
            for m in range(n):
                xc = s16.tile([128, PW], MMDT, tag="s16", name="xc")
                eng = nc.vector if (m % 2 == 0 or n <= 2) else nc.gpsimd
                eng.tensor_sub(xc, src[m], s1c)
                xcs.append(xc)
            m2 = scrw.tile([128, PW], FP, tag="scrw", name="m2")
            nc.vector.tensor_mul(m2, s1c, s1c)
            var = scrw.tile([128, PW], FP, tag="scrw", name="var")
            nc.vector.tensor_sub(var, s2c, m2)
            # rsqrt spline: 4.4e-5 max rel err, far below fp16 matmul noise,
            # so no Newton refinement is needed.
            rs = scrw.tile([128, PW], FP, tag="scrw", name="rs")
            act_rsqrt(rs, var)
            out = []
            for m in range(n):
                z = s16.tile([128, PW], MMDT, tag="s16", name="z")
                eng = nc.vector if (m % 2 == 0 or n <= 2) else nc.gpsimd
                eng.tensor_mul(z, xcs[m], rs)
                if dst is None:
                    d = s16.tile([128, PW], MMDT, tag="s16", name="lnout")
                else:
                    d = dst[m]
                nc.scalar.activation(
                    d, z,
                    AF.Gelu_apprx_tanh if fuse_gelu else AF.Identity,
                    bias=P(bkey, m), scale=P(gkey, m))
                out.append(d)
            return out

        def transformer_block(x, sfx):
            """x: 4 act tiles [128, chunk] fp16; returns ln2 output tiles.
            Emission is software-pipelined across the two pairs so the PE
            always has matmul work while an ln_pair chain runs."""
            wv = load_layer("w_v" + sfx, D0, D0)
            wo = load_layer("w_o" + sfx, D0, D0)
            f1 = load_layer("w_f1" + sfx, D0, DFF, pool=wide, tag="wide")
            f2 = load_layer("w_f2" + sfx, D0, DFF, pool=wide, tag="wide")
            f3 = load_layer("w_f3" + sfx, DFF, D0, pool=wf3p, tag="wf3")
            x2 = [act.tile([128, chunk], MMDT, tag="act", name="actb")
                  for _ in range(4)]
            st = [dict() for _ in range(npairs)]

            def st_vo(pr):
                po = pr * PW
                v = [s16.tile([128, PW], MMDT, tag="s16", name="v")
                     for _ in range(4)]
                for b2 in range(2):
                    co = po + b2 * BT
                    xs = [x[k][:, co:co + BT] for k in range(4)]
                    for m in range(4):
                        ps = pmm.tile([128, BT], FP, tag="mm", name="mm")
                        matmul_layer(wv, xs, m, 4, ps)
                        nc.scalar.activation(v[m][:, b2 * BT:(b2 + 1) * BT], ps,
                                             AF.Identity, bias=P("bv" + sfx, m))
                r1 = [s16.tile([128, PW], MMDT, tag="s16", name="r1")
                      for _ in range(4)]
                for b2 in range(2):
                    co = po + b2 * BT
                    vs = [v[k][:, b2 * BT:(b2 + 1) * BT] for k in range(4)]
                    for m in range(4):
                        ps = pmm.tile([128, BT], FP, tag="mm", name="mm")
                        matmul_layer(wo, vs, m, 4, ps)
                        stt(r1[m][:, b2 * BT:(b2 + 1) * BT], ps,
                            P("bo" + sfx, m), x[m][:, co:co + BT],
                            OP.add, OP.add)
                st[pr]["r1"] = r1

            def st_ln1(pr):
                st[pr]["x1"] = ln_pair(st[pr]["r1"], None, D0,
                                       "g1" + sfx, "be1" + sfx)

            def st_ffn(pr):
                po = pr * PW
                x1 = st[pr]["x1"]
                r2 = [s16.tile([128, PW], MMDT, tag="s16", name="r2")
                      for _ in range(4)]
                for b2 in range(2):
                    bsl = slice(b2 * BT, (b2 + 1) * BT)
                    xs = [x1[k][:, bsl] for k in range(4)]
                    hs = []
                    for ko in range(16):
                        pg = pgv.tile([128, BT], FP, tag="gv", name="gv")
                        matmul_layer(f1, xs, ko, 4, pg)
                        pv = pgv.tile([128, BT], FP, tag="gv", name="gv")
                        matmul_layer(f2, xs, ko, 4, pv)
                        vg = scrh.tile([128, BT], MMDT, tag="scrh", name="vg",
                                       bufs=20)
                        nc.scalar.activation(vg, pv, AF.Gelu_apprx_tanh,
                                             bias=P("bf2" + sfx, ko))
                        gsb = scrh.tile([128, BT], MMDT, tag="scrh", name="gsb",
                                        bufs=20)
                        nc.scalar.activation(gsb, pg, AF.Identity,
                                             bias=P("bf1" + sfx, ko))
                        h = scrh.tile([128, BT], MMDT, tag="scrh", name="h",
                                      bufs=20)
                        nc.gpsimd.tensor_mul(h, gsb, vg)
                        hs.append(h)
                    for m in range(4):
                        pf = pfa.tile([128, BT], FP, tag="fa", name="fa")
                        for ko in range(16):
                            nc.tensor.matmul(
                                pf, f3[ko][:, 128 * m:128 * (m + 1)], hs[ko],
                                start=(ko == 0), stop=(ko == 15))
                        stt(r2[m][:, bsl], pf, P("bf3" + sfx, m),
                            x1[m][:, bsl], OP.add, OP.add)
                st[pr]["r2"] = r2

            def st_ln2(pr):
                po = pr * PW
                ln_pair(st[pr]["r2"], [x2[m][:, po:po + PW] for m in range(4)],
                        D0, "g2" + sfx, "be2" + sfx)

            if npairs == 2:
                st_vo(0); st_ln1(0); st_vo(1); st_ffn(0)
                st_ln1(1); st_ln2(0); st_ffn(1); st_ln2(1)
            else:
                for pr in range(npairs):
                    st_vo(pr); st_ln1(pr); st_ffn(pr); st_ln2(pr)
            return x2

        def emit_s0(ci, w_in):
            """in_proj + in_norm for chunk ci; returns the 4 x-tiles."""
            cs = ci * chunk
            x = [act.tile([128, chunk], MMDT, tag="act", name="actb")
                 for _ in range(4)]

            def ip(pr):
                po = pr * PW
                t0 = [s16.tile([128, PW], MMDT, tag="s16", name="t0")
                      for _ in range(4)]
                for b2 in range(2):
                    c0 = cs + po + b2 * BT
                    ob = []
                    for k in range(3):
                        t = obsp.tile([128, BT], MMDT, tag="obs", name="obs")
                        nc.sync.dma_start(
                            out=t, in_=obsT[128 * k:128 * (k + 1), c0:c0 + BT])
                        ob.append(t)
                    for m in range(4):
                        ps = pmm.tile([128, BT], FP, tag="mm", name="mm")
                        matmul_layer(w_in, ob, m, 3, ps)
                        nc.scalar.activation(t0[m][:, b2 * BT:(b2 + 1) * BT],
                                             ps, AF.Identity, bias=P("b_in", m))
                return t0

            def ln0(pr, t0):
                po = pr * PW
                ln_pair(t0, [x[m][:, po:po + PW] for m in range(4)],
                        D0, "g_in", "be_in")
            return x, ip, ln0

        w_in0 = load_layer("w_in", OBS, D0)
        xn, ip_f, ln0_f = emit_s0(0, w_in0)
        t0a = ip_f(0); ln0_f(0, t0a)
        t0b = ip_f(1); ln0_f(1, t0b)
        s0_next = None

        for ci in range(nchunks):
            cs = ci * chunk
            x = xn

            x = transformer_block(x, "1")
            x = transformer_block(x, "2")

            # ---- hidden stage 0 (with residual) ----
            wh0 = load_layer("w_h0", D0, D0)
            wgt0 = load_layer("w_gt0", D0, 2 * D0, pool=wide, tag="wide")
            x5 = [act.tile([128, chunk], MMDT, tag="act", name="actb")
                  for _ in range(4)]
            hst = [dict() for _ in range(npairs)]

            def h0_u(pr):
                po = pr * PW
                tu = [s16.tile([128, PW], MMDT, tag="s16", name="tu")
                      for _ in range(4)]
                for b2 in range(2):
                    co = po + b2 * BT
                    xs = [x[k][:, co:co + BT] for k in range(4)]
                    for m in range(4):
                        ps = pmm.tile([128, BT], FP, tag="mm", name="mm")
                        matmul_layer(wh0, xs, m, 4, ps)
                        nc.scalar.activation(tu[m][:, b2 * BT:(b2 + 1) * BT],
                                             ps, AF.Identity, bias=P("bh0", m))
                hst[pr]["tu"] = tu

            def h0_ln(pr):
                hst[pr]["y"] = ln_pair(hst[pr]["tu"], None, D0,
                                       "gh0", "beh0", fuse_gelu=True)

            def h0_gate(pr):
                po = pr * PW
                y = hst[pr]["y"]
                for b2 in range(2):
                    co = po + b2 * BT
                    ys = [y[k][:, b2 * BT:(b2 + 1) * BT] for k in range(4)]
                    for m in range(4):
                        psg = pmm.tile([128, BT], FP, tag="mm", name="mm")
                        matmul_layer(wgt0, ys, m, 4, psg)
                        sg = scrh.tile([128, BT], FP, tag="scrhf", name="sg")
                        nc.scalar.activation(sg, psg, AF.Sigmoid,
                                             bias=P("bg0", m))
                        pt = pmm.tile([128, BT], FP, tag="mm", name="mm")
                        matmul_layer(wgt0, ys, m, 4, pt, cofs=D0)
                        a1 = scrh.tile([128, BT], FP, tag="scrhf", name="a1")
                        stt(a1, pt, P("bt0", m), ys[m], OP.add, OP.subtract)
                        a2 = scrh.tile([128, BT], FP, tag="scrhf", name="a2")
                        nc.vector.tensor_mul(a2, a1, sg)
                        a3 = scrh.tile([128, BT], FP, tag="scrhf", name="a3")
                        nc.vector.tensor_add(a3, a2, ys[m])
                        nc.vector.tensor_add(x5[m][:, co:co + BT], a3,
                                             x[m][:, co:co + BT])

            h0_sched = None  # emission interleaved with h1 below

            # ---- hidden stage 1 (no residual) + out ----
            wh1 = load_layer("w_h1", D0, DH1, pool=wsm2, tag="wsm2")
            wgt1 = load_layer("w_gt1", DH1, 2 * DH1, pool=wsm2, tag="wsm2")
            wout = load_layer("w_out", DH1, AOUT, pool=wsm2, tag="wsm2")
            h1st = [dict() for _ in range(npairs)]

            def h1_u(pr):
                po = pr * PW
                tu = [s16.tile([128, PW], MMDT, tag="s16", name="tu")
                      for _ in range(2)]
                for b2 in range(2):
                    co = po + b2 * BT
                    xs = [x5[k][:, co:co + BT] for k in range(4)]
                    for m in range(2):
                        ps = pmm.tile([128, BT], FP, tag="mm", name="mm")
                        matmul_layer(wh1, xs, m, 4, ps)
                        nc.scalar.activation(tu[m][:, b2 * BT:(b2 + 1) * BT],
                                             ps, AF.Identity, bias=P("bh1", m))
                h1st[pr]["tu"] = tu

            def h1_ln(pr):
                h1st[pr]["y"] = ln_pair(h1st[pr]["tu"], None, DH1,
                                        "gh1", "beh1", fuse_gelu=True)

            def h1_gate_out(pr):
                po = pr * PW
                y = h1st[pr]["y"]
                for b2 in range(2):
                    co = po + b2 * BT
                    ys = [y[k][:, b2 * BT:(b2 + 1) * BT] for k in range(2)]
                    x6 = []
                    for m in range(2):
                        psg = pmm.tile([128, BT], FP, tag="mm", name="mm")
                        matmul_layer(wgt1, ys, m, 2, psg)
                        sg = scrh.tile([128, BT], FP, tag="scrhf", name="sg")
                        nc.scalar.activation(sg, psg, AF.Sigmoid,
                                             bias=P("bg1", m))
                        pt = pmm.tile([128, BT], FP, tag="mm", name="mm")
                        matmul_layer(wgt1, ys, m, 2, pt, cofs=DH1)
                        a1 = scrh.tile([128, BT], FP, tag="scrhf", name="a1")
                        stt(a1, pt, P("bt1", m), ys[m], OP.add, OP.subtract)
                        a2 = scrh.tile([128, BT], FP, tag="scrhf", name="a2")
                        nc.vector.tensor_mul(a2, a1, sg)
                        a3 = scrh.tile([128, BT], MMDT, tag="scrh", name="a3",
                                       bufs=20)
                        nc.vector.tensor_add(a3, a2, ys[m])
                        x6.append(a3)
                    pso = pmm.tile([AOUT, BT], FP, tag="mm", name="mm")
                    for k in range(2):
                        nc.tensor.matmul(pso, wout[k][:, :AOUT], x6[k],
                                         start=(k == 0), stop=(k == 1))
                    ot = outp.tile([AOUT, BT], FP, tag="out", name="outt")
                    nc.scalar.activation(ot, pso, AF.Identity,
                                         bias=P("bout", 0)[:AOUT])
                    c0 = cs + po + b2 * BT
                    nc.sync.dma_start(out=out_d[:AOUT, c0:c0 + BT], in_=ot)

            nxt = None
            if ci + 1 < nchunks:
                w_inN = load_layer("w_in", OBS, D0)
                nxt = emit_s0(ci + 1, w_inN)
            if npairs == 2:
                h0_u(0); h0_ln(0); h0_u(1); h0_gate(0); h0_ln(1)
                if nxt is not None:
                    t0a = nxt[1](0)
                h1_u(0); h0_gate(1)
                if nxt is not None:
                    nxt[2](0, t0a)
                h1_ln(0)
                if nxt is not None:
                    t0b = nxt[1](1)
                h1_u(1); h1_gate_out(0)
                if nxt is not None:
                    nxt[2](1, t0b)
                h1_ln(1); h1_gate_out(1)
            else:
                for pr in range(npairs):
                    h0_u(pr); h0_ln(pr); h0_gate(pr)
                for pr in range(npairs):
                    h1_u(pr); h1_ln(pr); h1_gate_out(pr)
                if nxt is not None:
                    for pr in range(npairs):
                        t0x = nxt[1](pr); nxt[2](pr, t0x)
            if nxt is not None:
                xn = nxt[0]

    nc.compile()
    return nc


# ---------------------------------------------------------------------------
# host-side weight prep
# ---------------------------------------------------------------------------
def _f32(x):
    return np.ascontiguousarray(np.asarray(x), dtype=np.float32)


def _pack_params(p):
    cols = np.zeros((128, NCOL), np.float32)

    def put(key, vec):
        vec = _f32(vec).reshape(-1)
        d = dict(P_SPEC)[key]
        v = np.zeros(d, np.float32)
        v[:vec.shape[0]] = vec
        cols[:, P_COL[key]:P_COL[key] + d // 128] = v.reshape(-1, 128).T

    put("b_in", p["in_proj"]["b"])
    put("g_in", p["in_norm"]["g"])
    put("be_in", p["in_norm"]["b"])
    for i, sfx in enumerate(("1", "2")):
        blk = p["blocks"][i]
        put("bv" + sfx, blk["wv"]["b"])
        put("bo" + sfx, blk["wo"]["b"])
        put("g1" + sfx, blk["ln1"]["g"])
        put("be1" + sfx, blk["ln1"]["b"])
        put("bf1" + sfx, blk["ff1"]["b"])
        put("bf2" + sfx, blk["ff2"]["b"])
        put("bf3" + sfx, blk["ff3"]["b"])
        put("g2" + sfx, blk["ln2"]["g"])
        put("be2" + sfx, blk["ln2"]["b"])
    h0, h1 = p["hidden"][0], p["hidden"][1]
    put("bh0", h0["layer"]["b"])
    put("gh0", h0["norm"]["g"])
    put("beh0", h0["norm"]["b"])
    put("bg0", h0["gate"]["g"]["b"])
    put("bt0", h0["gate"]["t"]["b"])
    put("bh1", h1["layer"]["b"])
    put("gh1", h1["norm"]["g"])
    put("beh1", h1["norm"]["b"])
    put("bg1", h1["gate"]["g"]["b"])
    put("bt1", h1["gate"]["t"]["b"])
    put("bout", p["out"]["b"])
    return cols


def _net_weights(p):
    np_mm = np.float16 if MMDT == F16 else np.float32
    out = {}
    out["w_in"] = _f32(p["in_proj"]["w"]).T.copy()
    for i, sfx in enumerate(("1", "2")):
        blk = p["blocks"][i]
        out["w_v" + sfx] = _f32(blk["wv"]["w"]).T.copy()
        out["w_o" + sfx] = _f32(blk["wo"]["w"]).T.copy()
        out["w_f1" + sfx] = _f32(blk["ff1"]["w"]).T.copy()
        out["w_f2" + sfx] = _f32(blk["ff2"]["w"]).T.copy()
        out["w_f3" + sfx] = _f32(blk["ff3"]["w"]).T.copy()
    h0, h1 = p["hidden"][0], p["hidden"][1]
    out["w_h0"] = _f32(h0["layer"]["w"]).T.copy()
    out["w_gt0"] = np.concatenate(
        [_f32(h0["gate"]["g"]["w"]).T, _f32(h0["gate"]["t"]["w"]).T], axis=1).copy()
    out["w_h1"] = _f32(h1["layer"]["w"]).T.copy()
    out["w_gt1"] = np.concatenate(
        [_f32(h1["gate"]["g"]["w"]).T, _f32(h1["gate"]["t"]["w"]).T], axis=1).copy()
    w_out = _f32(p["out"]["w"]).T  # [256, odim]
    wo = np.zeros((DH1, AOUT), np.float32)
    wo[:, :w_out.shape[1]] = w_out
    out["w_out"] = wo
    return {k: np.ascontiguousarray(v, dtype=np_mm) for k, v in out.items()}


def make_in_maps(obs, actor_params, critic_params):
    obs = _f32(obs)
    aw = _net_weights(actor_params)
    ap = _pack_params(actor_params)
    cw = _net_weights(critic_params)
    cp = _pack_params(critic_params)
    in_maps = []
    for c in range(NCORES):
        s = (c % 4) * B_SHARD
        m = {"obsT": np.ascontiguousarray(obs[s:s + B_SHARD].T, dtype=np.float16 if MMDT == F16 else np.float32)}
        if c < 4:
            m.update(aw)
            m["params"] = ap
        else:
            m.update(cw)
            m["params"] = cp
        in_maps.append(m)
    return in_maps


_NC_CACHE = {}


def _get_nc():
    if "nc" not in _NC_CACHE:
        _NC_CACHE["nc"] = build_nc()
    return _NC_CACHE["nc"]


def run_on_hw(obs, actor_params, critic_params, trace=False, **kw):
    from concourse.bass_utils import run_bass_kernel_spmd
    nc = _get_nc()
    in_maps = make_in_maps(obs, actor_params, critic_params)
    res = run_bass_kernel_spmd(nc, in_maps, list(range(NCORES)), trace=trace, **kw)
    outs = [res.results[c]["out"] for c in range(NCORES)]
    action_mean = np.concatenate(outs[:4], axis=1).T.copy()        # [16384, 24]
    value = np.concatenate([o[:1] for o in outs[4:]], axis=1).T.copy()  # [16384, 1]
    return (action_mean, value), res


def kernel(obs, actor_params, critic_params):
    (action_mean, value), _ = run_on_hw(obs, actor_params, critic_params)
    return action_mean, value


# revision 29
# speedup vs baseline: 1.0783x; 1.0783x over previous
"""Trainium2 Bass kernel for nn_AdvancedActorCritic (dense_transformer).

Strategy:
  - 8 NeuronCores, SPMD: cores 0-3 run the actor net, cores 4-7 the critic
    net (its output layer zero-padded to the actor's shape so one program
    serves both).  Each core gets a 4096-row batch shard of obs.
  - seq_len == 1  =>  softmax over a single key is identity  =>  attention
    reduces to wo(wv(x)); wq/wk are dead code.
  - Activation layout: [features-on-partitions, batch-on-free].  Weights are
    pre-transposed on the host, so every linear layer is a weight-stationary
    matmul chain with N=512 moving tiles.
  - LayerNorm: partition-dim sums via ones-matmul on the PE (replicated over
    128 partitions), var = E[x^2]-m^2, ACT Sqrt + DVE reciprocal + one
    Newton-Raphson step for rsqrt, apply fused via scalar_tensor_tensor.
  - GELU: ACT Gelu_apprx_tanh (the exact tanh-approx formula of the ref).
"""

import sys

sys.path.insert(0, "/opt/trn_rl_repo")

from contextlib import ExitStack

import numpy as np

import concourse.bass as bass
import concourse.mybir as mybir
import concourse.tile as tile
from concourse import bacc

FP = mybir.dt.float32
F16 = mybir.dt.float16
MMDT = F16       # dtype for matmul operands (weights + activations)
AF = mybir.ActivationFunctionType
OP = mybir.AluOpType

OBS = 384
D0 = 512
DFF = 2048
DH1 = 256
AOUT = 24
EPS = 1e-5
BATCH = 16384
NCORES = 8
B_SHARD = BATCH // 4  # 4096 rows per core (actor on 0-3, critic on 4-7)
BT = 512              # matmul moving-tile width (fp32 PSUM bank)
CHUNK = 2048          # batch columns resident in SBUF per pass

# ---------------------------------------------------------------------------
# parameter-vector packing spec: (key, length).  Packed column-major into a
# [128, NCOL] f32 tensor; vector v occupies cols [off, off+len/128) with
# column j = v[128j:128j+128].
# ---------------------------------------------------------------------------
P_SPEC = [
    ("b_in", 512), ("g_in", 512), ("be_in", 512),
    ("bv1", 512), ("bo1", 512), ("g11", 512), ("be11", 512),
    ("bf11", 2048), ("bf21", 2048), ("bf31", 512), ("g21", 512), ("be21", 512),
    ("bv2", 512), ("bo2", 512), ("g12", 512), ("be12", 512),
    ("bf12", 2048), ("bf22", 2048), ("bf32", 512), ("g22", 512), ("be22", 512),
    ("bh0", 512), ("gh0", 512), ("beh0", 512), ("bg0", 512), ("bt0", 512),
    ("bh1", 256), ("gh1", 256), ("beh1", 256), ("bg1", 256), ("bt1", 256),
    ("bout", 128),
]
P_COL = {}
_c = 0
for _k, _d in P_SPEC:
    P_COL[_k] = _c
    _c += _d // 128
NCOL = _c

W_SHAPES = {
    "w_in": (OBS, D0),
    "w_v1": (D0, D0), "w_o1": (D0, D0),
    "w_f11": (D0, DFF), "w_f21": (D0, DFF), "w_f31": (DFF, D0),
    "w_v2": (D0, D0), "w_o2": (D0, D0),
    "w_f12": (D0, DFF), "w_f22": (D0, DFF), "w_f32": (DFF, D0),
    "w_h0": (D0, D0), "w_gt0": (D0, 2 * D0),
    "w_h1": (D0, DH1), "w_gt1": (DH1, 2 * DH1),
    "w_out": (DH1, AOUT),
}


# ---------------------------------------------------------------------------
# kernel program
# ---------------------------------------------------------------------------
def build_nc(b_shard=B_SHARD, chunk=CHUNK):
    nc = bacc.Bacc("TRN2", target_bir_lowering=False, debug=False)

    obsT = nc.dram_tensor("obsT", [OBS, b_shard], MMDT, kind="ExternalInput")
    wd = {
        name: nc.dram_tensor(name, list(shape), MMDT, kind="ExternalInput")
        for name, shape in W_SHAPES.items()
    }
    prm_d = nc.dram_tensor("params", [128, NCOL], FP, kind="ExternalInput")
    out_d = nc.dram_tensor("out", [AOUT, b_shard], FP, kind="ExternalOutput")

    PW = 2 * BT                 # LN batch width: a "pair" of matmul tiles
    nchunks = b_shard // chunk
    npairs = chunk // PW

    with tile.TileContext(nc) as tc, ExitStack() as ctx:
        act = ctx.enter_context(tc.tile_pool(name="act", bufs=9))
        wide = ctx.enter_context(tc.tile_pool(name="wide", bufs=8))
        wf3p = ctx.enter_context(tc.tile_pool(name="wf3p", bufs=16))
        wsm = ctx.enter_context(tc.tile_pool(name="wsm", bufs=8))
        wsm2 = ctx.enter_context(tc.tile_pool(name="wsm2", bufs=8))
        scrw = ctx.enter_context(tc.tile_pool(name="scrw", bufs=7))   # fp32 [128,PW]
        s16 = ctx.enter_context(tc.tile_pool(name="s16", bufs=18))     # fp16 [128,PW]
        scrh = ctx.enter_context(tc.tile_pool(name="scrh", bufs=6))   # fp16 [128,BT]
        obsp = ctx.enter_context(tc.tile_pool(name="obsp", bufs=4))
        cons = ctx.enter_context(tc.tile_pool(name="cons", bufs=1))
        outp = ctx.enter_context(tc.tile_pool(name="outp", bufs=1))
        pmm = ctx.enter_context(
            tc.tile_pool(name="pmm", bufs=2, space=bass.MemorySpace.PSUM))
        pgv = ctx.enter_context(
            tc.tile_pool(name="pgv", bufs=2, space=bass.MemorySpace.PSUM))
        pst = ctx.enter_context(
            tc.tile_pool(name="pst", bufs=2, space=bass.MemorySpace.PSUM))
        pfa = ctx.enter_context(
            tc.tile_pool(name="pfa", bufs=2, space=bass.MemorySpace.PSUM))

        ones = cons.tile([128, 128], MMDT, tag="ones", name="ones")
        nc.vector.memset(ones, 1.0)
        prm = cons.tile([128, NCOL], FP, tag="prm", name="prm")
        nc.sync.dma_start(out=prm, in_=prm_d[:, :])
        eps_t = cons.tile([128, 1], FP, tag="epsb", name="epsb")
        nc.vector.memset(eps_t, EPS)

        def P(key, m=0):
            c = P_COL[key] + m
            return prm[:, c:c + 1]

        def load_w(name, k, c0, cw, pool, tag):
            t = pool.tile([128, cw], MMDT, tag=tag, name="w_" + tag)
            nc.sync.dma_start(out=t, in_=wd[name][128 * k:128 * (k + 1), c0:c0 + cw])
            return t

        def load_layer(name, kdim, cw, pool=wsm, tag="wsm"):
            return [load_w(name, k, 0, cw, pool, tag) for k in range(kdim // 128)]

        def stt(out_ap, in0, scalar, in1, op0, op1):
            nc.vector.scalar_tensor_tensor(out_ap, in0, scalar, in1, op0, op1)

        def act_rsqrt(out_ap, in_ap):
            """out = 1/sqrt(in + EPS) via the ACT spline (direct emission —
            the bass wrapper refuses Rsqrt); one Newton step follows."""
            ins = [nc.scalar.lower_ap(in_ap),
                   nc.scalar.lower_ap(eps_t),
                   mybir.ImmediateValue(dtype=FP, value=1.0),
                   mybir.ImmediateValue(dtype=FP, value=0.0)]
            nc.scalar.add_instruction(
                mybir.InstActivation(
                    name=nc.get_next_instruction_name(),
                    func=AF.Rsqrt,
                    ins=ins,
                    outs=[nc.scalar.lower_ap(out_ap)]))

        def matmul_layer(w_tiles, src, m, n_k, out_ps, cofs=0):
            for k in range(n_k):
                nc.tensor.matmul(
                    out_ps, w_tiles[k][:, cofs + 128 * m: cofs + 128 * (m + 1)],
                    src[k], start=(k == 0), stop=(k == n_k - 1))

        def ln_pair(src, dst, D, gkey, bkey, fuse_gelu=False):
            """Batched layernorm over one pair (PW columns).
            src: list of [128, PW] APs (D//128).  dst: same, or None to
            allocate fp16 pair tiles internally (returned)."""
            n = D // 128
            sqs = []
            for m in range(n):
                sq = s16.tile([128, PW], MMDT, tag="s16", name="sq")
                nc.scalar.activation(sq, src[m], AF.Square)
                sqs.append(sq)
            s1c = scrw.tile([128, PW], FP, tag="scrw", name="s1c")
            s2c = scrw.tile([128, PW], FP, tag="scrw", name="s2c")
            for b2 in range(2):
                sl = slice(b2 * BT, (b2 + 1) * BT)
                ps1 = pst.tile([128, BT], FP, tag="st", name="st")
                for m in range(n):
                    nc.tensor.matmul(ps1, ones[:, :], src[m][:, sl],
                                     start=(m == 0), stop=(m == n - 1))
                nc.scalar.mul(s1c[:, sl], ps1, 1.0 / D)      # mean
                ps2 = pst.tile([128, BT], FP, tag="st", name="st")
                for m in range(n):
                    nc.tensor.matmul(ps2, ones[:, :], sqs[m][:, sl],
                                     start=(m == 0), stop=(m == n - 1))
                nc.scalar.mul(s2c[:, sl], ps2, 1.0 / D)      # E[x^2]
            # centered copies first: frees s1c/src before the rs chain ends.
            # Odd tiles go to GpSimd to keep the DVE chain short.
            xcs = []
            for m in range(n):
                xc = s16.tile([128, PW], MMDT, tag="s16", name="xc")
                nc.vector.tensor_sub(xc, src[m], s1c)
                xcs.append(xc)
            m2 = scrw.tile([128, PW], FP, tag="scrw", name="m2")
            nc.vector.tensor_mul(m2, s1c, s1c)
            var = scrw.tile([128, PW], FP, tag="scrw", name="var")
            nc.vector.tensor_sub(var, s2c, m2)
            # rsqrt spline: 4.4e-5 max rel err, far below fp16 matmul noise,
            # so no Newton refinement is needed.
            rs = scrw.tile([128, PW], FP, tag="scrw", name="rs")
            act_rsqrt(rs, var)
            out = []
            for m in range(n):
                z = s16.tile([128, PW], MMDT, tag="s16", name="z")
                nc.vector.tensor_mul(z, xcs[m], rs)
                if dst is None:
                    d = s16.tile([128, PW], MMDT, tag="s16", name="lnout")
                else:
                    d = dst[m]
                nc.scalar.activation(
                    d, z,
                    AF.Gelu_apprx_tanh if fuse_gelu else AF.Identity,
                    bias=P(bkey, m), scale=P(gkey, m))
                out.append(d)
            return out

        def transformer_block(x, sfx):
            """x: 4 act tiles [128, chunk] fp16; returns ln2 output tiles.
            Emission is software-pipelined across the two pairs so the PE
            always has matmul work while an ln_pair chain runs."""
            wv = load_layer("w_v" + sfx, D0, D0)
            wo = load_layer("w_o" + sfx, D0, D0)
            f1 = load_layer("w_f1" + sfx, D0, DFF, pool=wide, tag="wide")
            f2 = load_layer("w_f2" + sfx, D0, DFF, pool=wide, tag="wide")
            f3 = load_layer("w_f3" + sfx, DFF, D0, pool=wf3p, tag="wf3")
            x2 = [act.tile([128, chunk], MMDT, tag="act", name="actb")
                  for _ in range(4)]
            st = [dict() for _ in range(npairs)]

            def st_vo(pr):
                po = pr * PW
                v = [s16.tile([128, PW], MMDT, tag="s16", name="v")
                     for _ in range(4)]
                for b2 in range(2):
                    co = po + b2 * BT
                    xs = [x[k][:, co:co + BT] for k in range(4)]
                    for m in range(4):
                        ps = pmm.tile([128, BT], FP, tag="mm", name="mm")
                        matmul_layer(wv, xs, m, 4, ps)
                        nc.scalar.activation(v[m][:, b2 * BT:(b2 + 1) * BT], ps,
                                             AF.Identity, bias=P("bv" + sfx, m))
                r1 = [s16.tile([128, PW], MMDT, tag="s16", name="r1")
                      for _ in range(4)]
                for b2 in range(2):
                    co = po + b2 * BT
                    vs = [v[k][:, b2 * BT:(b2 + 1) * BT] for k in range(4)]
                    for m in range(4):
                        ps = pmm.tile([128, BT], FP, tag="mm", name="mm")
                        matmul_layer(wo, vs, m, 4, ps)
                        stt(r1[m][:, b2 * BT:(b2 + 1) * BT], ps,
                            P("bo" + sfx, m), x[m][:, co:co + BT],
                            OP.add, OP.add)
                st[pr]["r1"] = r1

            def st_ln1(pr):
                st[pr]["x1"] = ln_pair(st[pr]["r1"], None, D0,
                                       "g1" + sfx, "be1" + sfx)

            def st_ffn(pr):
                po = pr * PW
                x1 = st[pr]["x1"]
                r2 = [s16.tile([128, PW], MMDT, tag="s16", name="r2")
                      for _ in range(4)]
                for b2 in range(2):
                    bsl = slice(b2 * BT, (b2 + 1) * BT)
                    xs = [x1[k][:, bsl] for k in range(4)]
                    hs = []
                    for ko in range(16):
                        pg = pgv.tile([128, BT], FP, tag="gv", name="gv")
                        matmul_layer(f1, xs, ko, 4, pg)
                        pv = pgv.tile([128, BT], FP, tag="gv", name="gv")
                        matmul_layer(f2, xs, ko, 4, pv)
                        vg = scrh.tile([128, BT], MMDT, tag="scrh", name="vg",
                                       bufs=20)
                        nc.scalar.activation(vg, pv, AF.Gelu_apprx_tanh,
                                             bias=P("bf2" + sfx, ko))
                        gsb = scrh.tile([128, BT], MMDT, tag="scrh", name="gsb",
                                        bufs=20)
                        nc.scalar.activation(gsb, pg, AF.Identity,
                                             bias=P("bf1" + sfx, ko))
                        h = scrh.tile([128, BT], MMDT, tag="scrh", name="h",
                                      bufs=20)
                        nc.gpsimd.tensor_mul(h, gsb, vg)
                        hs.append(h)
                    for m in range(4):
                        pf = pfa.tile([128, BT], FP, tag="fa", name="fa")
                        for ko in range(16):
                            nc.tensor.matmul(
                                pf, f3[ko][:, 128 * m:128 * (m + 1)], hs[ko],
                                start=(ko == 0), stop=(ko == 15))
                        stt(r2[m][:, bsl], pf, P("bf3" + sfx, m),
                            x1[m][:, bsl], OP.add, OP.add)
                st[pr]["r2"] = r2

            def st_ln2(pr):
                po = pr * PW
                ln_pair(st[pr]["r2"], [x2[m][:, po:po + PW] for m in range(4)],
                        D0, "g2" + sfx, "be2" + sfx)

            if npairs == 2:
                st_vo(0); st_ln1(0); st_vo(1); st_ffn(0)
                st_ln1(1); st_ln2(0); st_ffn(1); st_ln2(1)
            else:
                for pr in range(npairs):
                    st_vo(pr); st_ln1(pr); st_ffn(pr); st_ln2(pr)
            return x2

        def emit_s0(ci, w_in):
            """in_proj + in_norm for chunk ci; returns the 4 x-tiles."""
            cs = ci * chunk
            x = [act.tile([128, chunk], MMDT, tag="act", name="actb")
                 for _ in range(4)]

            def ip(pr):
                po = pr * PW
                t0 = [s16.tile([128, PW], MMDT, tag="s16", name="t0")
                      for _ in range(4)]
                for b2 in range(2):
                    c0 = cs + po + b2 * BT
                    ob = []
                    for k in range(3):
                        t = obsp.tile([128, BT], MMDT, tag="obs", name="obs")
                        nc.sync.dma_start(
                            out=t, in_=obsT[128 * k:128 * (k + 1), c0:c0 + BT])
                        ob.append(t)
                    for m in range(4):
                        ps = pmm.tile([128, BT], FP, tag="mm", name="mm")
                        matmul_layer(w_in, ob, m, 3, ps)
                        nc.scalar.activation(t0[m][:, b2 * BT:(b2 + 1) * BT],
                                             ps, AF.Identity, bias=P("b_in", m))
                return t0

            def ln0(pr, t0):
                po = pr * PW
                ln_pair(t0, [x[m][:, po:po + PW] for m in range(4)],
                        D0, "g_in", "be_in")
            return x, ip, ln0

        w_in0 = load_layer("w_in", OBS, D0)
        xn, ip_f, ln0_f = emit_s0(0, w_in0)
        t0a = ip_f(0); ln0_f(0, t0a)
        t0b = ip_f(1); ln0_f(1, t0b)
        s0_next = None

        for ci in range(nchunks):
            cs = ci * chunk
            x = xn

            x = transformer_block(x, "1")
            x = transformer_block(x, "2")

            # ---- hidden stage 0 (with residual) ----
            wh0 = load_layer("w_h0", D0, D0)
            wgt0 = load_layer("w_gt0", D0, 2 * D0, pool=wide, tag="wide")
            x5 = [act.tile([128, chunk], MMDT, tag="act", name="actb")
                  for _ in range(4)]
            hst = [dict() for _ in range(npairs)]

            def h0_u(pr):
                po = pr * PW
                tu = [s16.tile([128, PW], MMDT, tag="s16", name="tu")
                      for _ in range(4)]
                for b2 in range(2):
                    co = po + b2 * BT
                    xs = [x[k][:, co:co + BT] for k in range(4)]
                    for m in range(4):
                        ps = pmm.tile([128, BT], FP, tag="mm", name="mm")
                        matmul_layer(wh0, xs, m, 4, ps)
                        nc.scalar.activation(tu[m][:, b2 * BT:(b2 + 1) * BT],
                                             ps, AF.Identity, bias=P("bh0", m))
                hst[pr]["tu"] = tu

            def h0_ln(pr):
                hst[pr]["y"] = ln_pair(hst[pr]["tu"], None, D0,
                                       "gh0", "beh0", fuse_gelu=True)

            def h0_gate(pr):
                po = pr * PW
                y = hst[pr]["y"]
                for b2 in range(2):
                    co = po + b2 * BT
                    ys = [y[k][:, b2 * BT:(b2 + 1) * BT] for k in range(4)]
                    for m in range(4):
                        psg = pmm.tile([128, BT], FP, tag="mm", name="mm")
                        matmul_layer(wgt0, ys, m, 4, psg)
                        sg = scrh.tile([128, BT], FP, tag="scrhf", name="sg")
                        nc.scalar.activation(sg, psg, AF.Sigmoid,
                                             bias=P("bg0", m))
                        pt = pmm.tile([128, BT], FP, tag="mm", name="mm")
                        matmul_layer(wgt0, ys, m, 4, pt, cofs=D0)
                        a1 = scrh.tile([128, BT], FP, tag="scrhf", name="a1")
                        stt(a1, pt, P("bt0", m), ys[m], OP.add, OP.subtract)
                        a2 = scrh.tile([128, BT], FP, tag="scrhf", name="a2")
                        nc.vector.tensor_mul(a2, a1, sg)
                        a3 = scrh.tile([128, BT], FP, tag="scrhf", name="a3")
                        nc.vector.tensor_add(a3, a2, ys[m])
                        nc.vector.tensor_add(x5[m][:, co:co + BT], a3,
                                             x[m][:, co:co + BT])

            h0_sched = None  # emission interleaved with h1 below

            # ---- hidden stage 1 (no residual) + out ----
            wh1 = load_layer("w_h1", D0, DH1, pool=wsm2, tag="wsm2")
            wgt1 = load_layer("w_gt1", DH1, 2 * DH1, pool=wsm2, tag="wsm2")
            wout = load_layer("w_out", DH1, AOUT, pool=wsm2, tag="wsm2")
            h1st = [dict() for _ in range(npairs)]

            def h1_u(pr):
                po = pr * PW
                tu = [s16.tile([128, PW], MMDT, tag="s16", name="tu")
                      for _ in range(2)]
                for b2 in range(2):
                    co = po + b2 * BT
                    xs = [x5[k][:, co:co + BT] for k in range(4)]
                    for m in range(2):
                        ps = pmm.tile([128, BT], FP, tag="mm", name="mm")
                        matmul_layer(wh1, xs, m, 4, ps)
                        nc.scalar.activation(tu[m][:, b2 * BT:(b2 + 1) * BT],
                                             ps, AF.Identity, bias=P("bh1", m))
                h1st[pr]["tu"] = tu

            def h1_ln(pr):
                h1st[pr]["y"] = ln_pair(h1st[pr]["tu"], None, DH1,
                                        "gh1", "beh1", fuse_gelu=True)

            def h1_gate_out(pr):
                po = pr * PW
                y = h1st[pr]["y"]
                for b2 in range(2):
                    co = po + b2 * BT
                    ys = [y[k][:, b2 * BT:(b2 + 1) * BT] for k in range(2)]
                    x6 = []
                    for m in range(2):
                        psg = pmm.tile([128, BT], FP, tag="mm", name="mm")
                        matmul_layer(wgt1, ys, m, 2, psg)
                        sg = scrh.tile([128, BT], FP, tag="scrhf", name="sg")
                        nc.scalar.activation(sg, psg, AF.Sigmoid,
                                             bias=P("bg1", m))
                        pt = pmm.tile([128, BT], FP, tag="mm", name="mm")
                        matmul_layer(wgt1, ys, m, 2, pt, cofs=DH1)
                        a1 = scrh.tile([128, BT], FP, tag="scrhf", name="a1")
                        stt(a1, pt, P("bt1", m), ys[m], OP.add, OP.subtract)
                        a2 = scrh.tile([128, BT], FP, tag="scrhf", name="a2")
                        nc.vector.tensor_mul(a2, a1, sg)
                        a3 = scrh.tile([128, BT], MMDT, tag="scrh", name="a3",
                                       bufs=20)
                        nc.vector.tensor_add(a3, a2, ys[m])
                        x6.append(a3)
                    pso = pmm.tile([AOUT, BT], FP, tag="mm", name="mm")
                    for k in range(2):
                        nc.tensor.matmul(pso, wout[k][:, :AOUT], x6[k],
                                         start=(k == 0), stop=(k == 1))
                    ot = outp.tile([AOUT, BT], FP, tag="out", name="outt")
                    nc.scalar.activation(ot, pso, AF.Identity,
                                         bias=P("bout", 0)[:AOUT])
                    c0 = cs + po + b2 * BT
                    nc.sync.dma_start(out=out_d[:AOUT, c0:c0 + BT], in_=ot)

            nxt = None
            if ci + 1 < nchunks:
                w_inN = load_layer("w_in", OBS, D0)
                nxt = emit_s0(ci + 1, w_inN)
            if npairs == 2:
                h0_u(0); h0_ln(0); h0_u(1); h0_gate(0); h0_ln(1)
                if nxt is not None:
                    t0a = nxt[1](0)
                h1_u(0); h0_gate(1)
                if nxt is not None:
                    nxt[2](0, t0a)
                h1_ln(0)
                if nxt is not None:
                    t0b = nxt[1](1)
                h1_u(1); h1_gate_out(0)
                if nxt is not None:
                    nxt[2](1, t0b)
                h1_ln(1); h1_gate_out(1)
            else:
                for pr in range(npairs):
                    h0_u(pr); h0_ln(pr); h0_gate(pr)
                for pr in range(npairs):
                    h1_u(pr); h1_ln(pr); h1_gate_out(pr)
                if nxt is not None:
                    for pr in range(npairs):
                        t0x = nxt[1](pr); nxt[2](pr, t0x)
            if nxt is not None:
                xn = nxt[0]

    nc.compile()
    return nc


# ---------------------------------------------------------------------------
# host-side weight prep
# ---------------------------------------------------------------------------
def _f32(x):
    return np.ascontiguousarray(np.asarray(x), dtype=np.float32)


def _pack_params(p):
    cols = np.zeros((128, NCOL), np.float32)

    def put(key, vec):
        vec = _f32(vec).reshape(-1)
        d = dict(P_SPEC)[key]
        v = np.zeros(d, np.float32)
        v[:vec.shape[0]] = vec
        cols[:, P_COL[key]:P_COL[key] + d // 128] = v.reshape(-1, 128).T

    put("b_in", p["in_proj"]["b"])
    put("g_in", p["in_norm"]["g"])
    put("be_in", p["in_norm"]["b"])
    for i, sfx in enumerate(("1", "2")):
        blk = p["blocks"][i]
        put("bv" + sfx, blk["wv"]["b"])
        put("bo" + sfx, blk["wo"]["b"])
        put("g1" + sfx, blk["ln1"]["g"])
        put("be1" + sfx, blk["ln1"]["b"])
        put("bf1" + sfx, blk["ff1"]["b"])
        put("bf2" + sfx, blk["ff2"]["b"])
        put("bf3" + sfx, blk["ff3"]["b"])
        put("g2" + sfx, blk["ln2"]["g"])
        put("be2" + sfx, blk["ln2"]["b"])
    h0, h1 = p["hidden"][0], p["hidden"][1]
    put("bh0", h0["layer"]["b"])
    put("gh0", h0["norm"]["g"])
    put("beh0", h0["norm"]["b"])
    put("bg0", h0["gate"]["g"]["b"])
    put("bt0", h0["gate"]["t"]["b"])
    put("bh1", h1["layer"]["b"])
    put("gh1", h1["norm"]["g"])
    put("beh1", h1["norm"]["b"])
    put("bg1", h1["gate"]["g"]["b"])
    put("bt1", h1["gate"]["t"]["b"])
    put("bout", p["out"]["b"])
    return cols


def _net_weights(p):
    np_mm = np.float16 if MMDT == F16 else np.float32
    out = {}
    out["w_in"] = _f32(p["in_proj"]["w"]).T.copy()
    for i, sfx in enumerate(("1", "2")):
        blk = p["blocks"][i]
        out["w_v" + sfx] = _f32(blk["wv"]["w"]).T.copy()
        out["w_o" + sfx] = _f32(blk["wo"]["w"]).T.copy()
        out["w_f1" + sfx] = _f32(blk["ff1"]["w"]).T.copy()
        out["w_f2" + sfx] = _f32(blk["ff2"]["w"]).T.copy()
        out["w_f3" + sfx] = _f32(blk["ff3"]["w"]).T.copy()
    h0, h1 = p["hidden"][0], p["hidden"][1]
    out["w_h0"] = _f32(h0["layer"]["w"]).T.copy()
    out["w_gt0"] = np.concatenate(
        [_f32(h0["gate"]["g"]["w"]).T, _f32(h0["gate"]["t"]["w"]).T], axis=1).copy()
    out["w_h1"] = _f32(h1["layer"]["w"]).T.copy()
    out["w_gt1"] = np.concatenate(
        [_f32(h1["gate"]["g"]["w"]).T, _f32(h1["gate"]["t"]["w"]).T], axis=1).copy()
    w_out = _f32(p["out"]["w"]).T  # [256, odim]
    wo = np.zeros((DH1, AOUT), np.float32)
    wo[:, :w_out.shape[1]] = w_out
    out["w_out"] = wo
    return {k: np.ascontiguousarray(v, dtype=np_mm) for k, v in out.items()}


def make_in_maps(obs, actor_params, critic_params):
    obs = _f32(obs)
    aw = _net_weights(actor_params)
    ap = _pack_params(actor_params)
    cw = _net_weights(critic_params)
    cp = _pack_params(critic_params)
    in_maps = []
    for c in range(NCORES):
        s = (c % 4) * B_SHARD
        m = {"obsT": np.ascontiguousarray(obs[s:s + B_SHARD].T, dtype=np.float16 if MMDT == F16 else np.float32)}
        if c < 4:
            m.update(aw)
            m["params"] = ap
        else:
            m.update(cw)
            m["params"] = cp
        in_maps.append(m)
    return in_maps


_NC_CACHE = {}


def _get_nc():
    if "nc" not in _NC_CACHE:
        _NC_CACHE["nc"] = build_nc()
    return _NC_CACHE["nc"]


def run_on_hw(obs, actor_params, critic_params, trace=False, **kw):
    from concourse.bass_utils import run_bass_kernel_spmd
    nc = _get_nc()
    in_maps = make_in_maps(obs, actor_params, critic_params)
    res = run_bass_kernel_spmd(nc, in_maps, list(range(NCORES)), trace=trace, **kw)
    outs = [res.results[c]["out"] for c in range(NCORES)]
    action_mean = np.concatenate(outs[:4], axis=1).T.copy()        # [16384, 24]
    value = np.concatenate([o[:1] for o in outs[4:]], axis=1).T.copy()  # [16384, 1]
    return (action_mean, value), res


def kernel(obs, actor_params, critic_params):
    (action_mean, value), _ = run_on_hw(obs, actor_params, critic_params)
    return action_mean, value
